# revision 2
# baseline (speedup 1.0000x reference)
"""Trainium2 Bass kernel for nn_ComponentIoULoss.

Loss = global BCE (label smoothing, pos_weight=1) + mean per-connected-component
crop-dice over dilated component bboxes.

Device strategy (8 NeuronCores, data-parallel over batch, 2 images/core):
  - BCE partial sums on-device: sum(x), sum(t*x), sum(ln sigmoid(x))
    (softplus(-x) = -ln(sigmoid(x)); BCE elem = (1-t')x + softplus(-x)).
  - Per-component crop dice needs box sums of p=sigmoid(x) and p*m over
    dilated component bboxes.  Targets are 8x8-blocky, so dilated bbox
    corners can only take 128 distinct values per axis:
    {0} + {8k-3 : k=1..63} + {8m+3 : m=1..63} + {512}.
    The device computes a "selected summed-area table": T1 = Lsel^T @ A
    (Lsel [512 x 128] prefix-indicator matrix, A in {p*m, p}), then a
    cumulative-sum scan along the free axis, and DMAs out the 128 selected
    columns -> SAT[128 x 128] per image per matrix.  The host reads 4 SAT
    corners per component (exact prefix-sum differences).
  - Connected-component labeling (tiny integer problem, 64x64 coarse grid
    per image) and the final scalar combine run on the host.
"""

import sys

if "/opt/trn_rl_repo" not in sys.path:
    sys.path.insert(0, "/opt/trn_rl_repo")

import numpy as np
import ml_dtypes

N_CORES = 8
B, H, W = 16, 512, 512
BPC = B // N_CORES  # batches per core
BLK = 8
NC_COARSE = H // BLK  # 64
DIL = 3
EPS = 0.05
NSEL = 128

_COMPILED = {}


# ---------------------------------------------------------------------------
# selector geometry
# ---------------------------------------------------------------------------

def _sel_values():
    """The 128 prefix lengths v such that SAT[q] = sum over rows i < v[q]."""
    vs = [0]
    vs += [8 * k - 3 for k in range(1, 64)]
    vs += [8 * m + 3 for m in range(1, 64)]
    vs += [512]
    return np.array(vs, dtype=np.int64)


def _lo_index(r0):
    """Selector index for a box low edge r0 in {0} | {8k-3}."""
    return np.where(r0 == 0, 0, (r0 + 3) // 8)


def _hi_index(r1):
    """Selector index for a box high edge r1 in {8m+3} | {512}."""
    return np.where(r1 == 512, 127, 63 + (r1 - 3) // 8)


def _lsel_host():
    """Lsel^T [512 x 128] bf16 as [4, 128, 128] contract-chunk-major."""
    vs = _sel_values()
    i = np.arange(512)
    m = (i[:, None] < vs[None, :]).astype(ml_dtypes.bfloat16)
    return np.ascontiguousarray(m.reshape(4, 128, 128))


# ---------------------------------------------------------------------------
# host connected components (min-index propagation, mirrors reference)
# ---------------------------------------------------------------------------

def _cc_boxes(mask):
    """4-connected components of bool [h, w]. Returns (rmin, rmax, cmin, cmax)
    int arrays, one entry per component."""
    h, w = mask.shape
    n = h * w
    idx = np.arange(n, dtype=np.int64).reshape(h, w)
    lab = np.where(mask, idx, n)
    big = n
    while True:
        p = np.pad(lab, 1, constant_values=big)
        nmin = np.minimum(
            np.minimum(p[:-2, 1:-1], p[2:, 1:-1]),
            np.minimum(p[1:-1, :-2], p[1:-1, 2:]),
        )
        l2 = np.where(mask, np.minimum(lab, nmin), big)
        flat = np.append(l2.ravel(), big)
        flat = flat[flat]
        flat = flat[flat]
        l2 = flat[:-1].reshape(h, w)
        if np.array_equal(l2, lab):
            break
        lab = l2
    rows, cols = np.nonzero(mask)
    labs = lab[mask]
    uniq, inv = np.unique(labs, return_inverse=True)
    k = len(uniq)
    rmin = np.full(k, h, dtype=np.int64)
    rmax = np.full(k, -1, dtype=np.int64)
    cmin = np.full(k, w, dtype=np.int64)
    cmax = np.full(k, -1, dtype=np.int64)
    np.minimum.at(rmin, inv, rows)
    np.maximum.at(rmax, inv, rows)
    np.minimum.at(cmin, inv, cols)
    np.maximum.at(cmax, inv, cols)
    return rmin, rmax, cmin, cmax


# ---------------------------------------------------------------------------
# device kernel
# ---------------------------------------------------------------------------

def _build():
    import concourse.bacc as bacc
    import concourse.tile as tile
    from concourse import mybir
    from concourse.bass import _add_dep_helper

    F32 = mybir.dt.float32
    BF16 = mybir.dt.bfloat16
    AF = mybir.ActivationFunctionType
    OP = mybir.AluOpType

    nc = bacc.Bacc("TRN2", target_bir_lowering=False, debug=False,
                   num_devices=N_CORES)

    logit_d = nc.dram_tensor("logit", [BPC, H, W], F32, kind="ExternalInput")
    target_d = nc.dram_tensor("target", [BPC, H, W], F32, kind="ExternalInput")
    lsel_d = nc.dram_tensor("lsel", [4, 128, 128], BF16, kind="ExternalInput")
    sat_pm_d = nc.dram_tensor("sat_pm", [BPC, 128, 128], F32, kind="ExternalOutput")
    sat_p_d = nc.dram_tensor("sat_p", [BPC, 128, 128], F32, kind="ExternalOutput")
    accs_d = nc.dram_tensor("accs", [3 * BPC, 128, 1], F32, kind="ExternalOutput")

    with tile.TileContext(nc) as tc:
        with (
            tc.tile_pool(name="io", bufs=2) as io_pool,
            tc.tile_pool(name="mid", bufs=2) as mid_pool,
            tc.tile_pool(name="scr", bufs=2) as scr_pool,
            tc.tile_pool(name="scan", bufs=4) as scan_pool,
            tc.tile_pool(name="small", bufs=1) as small_pool,
            tc.tile_pool(name="ps", bufs=1, space="PSUM") as ps_pool,
        ):
            lsel = small_pool.tile([128, 4, 128], BF16, tag="lsel")
            nc.sync.dma_start(
                lsel[:], lsel_d.ap().rearrange("c p q -> p c q")
            )
            dummy = small_pool.tile([128, 512], F32, tag="dummy")
            nc.gpsimd.memset(dummy[:], 0.0)

            xs, ts_, sig_insts = [], [], []
            for b in range(BPC):
                x = io_pool.tile([128, 2048], F32, tag=f"x{b}")
                nc.sync.dma_start(
                    x[:].rearrange("p (c w) -> p c w", w=W),
                    logit_d.ap()[b].rearrange("(c p) w -> p c w", p=128),
                )
                t = io_pool.tile([128, 2048], F32, tag=f"t{b}")
                nc.sync.dma_start(
                    t[:].rearrange("p (c w) -> p c w", w=W),
                    target_d.ap()[b].rearrange("(c p) w -> p c w", p=128),
                )
                xs.append(x)
                ts_.append(t)

            p_bfs = []
            for b in range(BPC):
                p_bf = mid_pool.tile([128, 2048], BF16, tag=f"p{b}")
                sig_insts.append(
                    nc.scalar.activation(p_bf[:], xs[b][:], AF.Sigmoid)
                )
                p_bfs.append(p_bf)

            for b in range(BPC):
                x, t, p_bf = xs[b], ts_[b], p_bfs[b]

                # --- BCE partial sums ---
                acc_ln = small_pool.tile([128, 1], F32, tag=f"acc_ln{b}")
                lnscr = scr_pool.tile([128, 2048], BF16, tag=f"lnscr{b}")
                ln_inst = nc.scalar.activation(
                    lnscr[:], p_bf[:], AF.Ln, accum_out=acc_ln[:]
                )
                # keep ACT ordered sigmoid,sigmoid,ln,ln -> 2 act-table loads
                _add_dep_helper(ln_inst.ins, sig_insts[-1].ins, sync=False,
                                reason="group act tables")
                nc.sync.dma_start(accs_d.ap()[b], acc_ln[:])

                acc_x = small_pool.tile([128, 1], F32, tag=f"acc_x{b}")
                xscr = scr_pool.tile([128, 2048], BF16, tag=f"xscr{b}")
                nc.vector.tensor_scalar(
                    xscr[:], x[:], 1.0, None, OP.mult, OP.add,
                    accum_out=acc_x[:],
                )
                nc.sync.dma_start(accs_d.ap()[BPC + b], acc_x[:])

                tx = scr_pool.tile([128, 2048], BF16, tag=f"tx{b}")
                nc.vector.tensor_tensor(tx[:], t[:], x[:], OP.mult)
                acc_tx = small_pool.tile([128, 1], F32, tag=f"acc_tx{b}")
                txscr = scr_pool.tile([128, 2048], BF16, tag=f"txscr{b}")
                nc.vector.tensor_scalar(
                    txscr[:], tx[:], 1.0, None, OP.mult, OP.add,
                    accum_out=acc_tx[:],
                )
                nc.sync.dma_start(accs_d.ap()[2 * BPC + b], acc_tx[:])

                # --- a1 = p * m (exact: m is 0/1) ---
                a1 = mid_pool.tile([128, 2048], BF16, tag=f"a1{b}")
                nc.gpsimd.tensor_tensor(a1[:], p_bf[:], t[:], OP.mult)

                # --- selected SAT for each matrix ---
                for name, rhs, out_d in (
                    ("pm", a1, sat_pm_d),
                    ("p", p_bf, sat_p_d),
                ):
                    t1 = ps_pool.tile([128, 512], F32, tag=f"t1{name}{b}")
                    rhs3 = rhs[:].rearrange("p (c w) -> p c w", w=W)
                    for c in range(4):
                        nc.tensor.matmul(
                            t1[:], lsel[:, c, :], rhs3[:, c, :],
                            start=(c == 0), stop=(c == 3),
                        )
                    scan = scan_pool.tile([128, 512], F32, tag=f"scan{name}{b}")
                    nc.vector.tensor_tensor_scan(
                        scan[:], t1[:], dummy[:], 0.0, OP.add, OP.add
                    )
                    od = out_d.ap()[b]
                    nc.sync.dma_start(od[:, 1:64], scan[:, 4:508:8])
                    nc.sync.dma_start(od[:, 64:127], scan[:, 10:512:8])
                    nc.sync.dma_start(od[:, 127:128], scan[:, 511:512])

    nc.compile()
    return nc


def _get_compiled():
    if "nc" not in _COMPILED:
        _COMPILED["nc"] = _build()
    return _COMPILED["nc"]


# ---------------------------------------------------------------------------
# host fallback (general inputs; not hit for blocky targets)
# ---------------------------------------------------------------------------

def _host_reference(logit, target):
    x = logit[:, 0].astype(np.float64)
    t = target[:, 0].astype(np.float64)
    ts = t * (1.0 - EPS) + (1.0 - t) * EPS
    bce = np.mean((1.0 - ts) * x + np.logaddexp(0.0, -x))
    p = 1.0 / (1.0 + np.exp(-x))
    tot_sum = 0.0
    tot_cnt = 0
    for b in range(B):
        mask = t[b] > 0.5
        if not mask.any():
            continue
        rmin, rmax, cmin, cmax = _cc_boxes(mask)
        r0 = np.clip(rmin - DIL, 0, H)
        r1 = np.clip(rmax + 1 + DIL, 0, H)
        c0 = np.clip(cmin - DIL, 0, W)
        c1 = np.clip(cmax + 1 + DIL, 0, W)
        sat_pm = np.zeros((H + 1, W + 1))
        sat_pm[1:, 1:] = (p[b] * t[b]).cumsum(0).cumsum(1)
        sat_p = np.zeros((H + 1, W + 1))
        sat_p[1:, 1:] = p[b].cumsum(0).cumsum(1)
        sat_m = np.zeros((H + 1, W + 1))
        sat_m[1:, 1:] = t[b].cumsum(0).cumsum(1)

        def box(s):
            return s[r1, c1] - s[r0, c1] - s[r1, c0] + s[r0, c0]

        dice = (2.0 * box(sat_pm) + 1.0) / (box(sat_p) + box(sat_m) + 1.0)
        tot_sum += np.sum(1.0 - dice)
        tot_cnt += len(dice)
    l_comp = tot_sum / max(tot_cnt, 1) if tot_cnt > 0 else 0.0
    return np.array(bce + l_comp, dtype=np.float32)


# ---------------------------------------------------------------------------
# entry point
# ---------------------------------------------------------------------------

def kernel(logit, target):
    from concourse.bass_utils import run_bass_kernel_spmd

    logit = np.ascontiguousarray(np.asarray(logit, dtype=np.float32))
    target = np.ascontiguousarray(np.asarray(target, dtype=np.float32))
    assert logit.shape == (B, 1, H, W) and target.shape == (B, 1, H, W)

    tgt = target[:, 0]
    coarse = tgt[:, ::BLK, ::BLK]
    blocky = np.array_equal(
        tgt.reshape(B, NC_COARSE, BLK, NC_COARSE, BLK),
        np.broadcast_to(
            coarse[:, :, None, :, None],
            (B, NC_COARSE, BLK, NC_COARSE, BLK),
        ),
    )
    if not blocky:
        return _host_reference(logit, target)

    nc = _get_compiled()
    lsel = _lsel_host()
    in_maps = [
        {
            "logit": np.ascontiguousarray(logit[c * BPC:(c + 1) * BPC, 0]),
            "target": np.ascontiguousarray(target[c * BPC:(c + 1) * BPC, 0]),
            "lsel": lsel,
        }
        for c in range(N_CORES)
    ]
    res = run_bass_kernel_spmd(nc, in_maps, core_ids=list(range(N_CORES)))

    # ---- BCE ----
    s_ln = 0.0
    s_x = 0.0
    s_tx = 0.0
    for c in range(N_CORES):
        accs = res.results[c]["accs"].astype(np.float64)
        s_ln += accs[0:BPC].sum()
        s_x += accs[BPC:2 * BPC].sum()
        s_tx += accs[2 * BPC:3 * BPC].sum()
    n_tot = float(B * H * W)
    # sum softplus(-x) = -sum ln p ; bce elem = (1-t')x + softplus(-x)
    bce = ((1.0 - EPS) * s_x - (1.0 - 2.0 * EPS) * s_tx - s_ln) / n_tot

    # ---- per-component crop dice ----
    tm = tgt > 0.5
    sat_m = np.zeros((B, H + 1, W + 1), dtype=np.int64)
    sat_m[:, 1:, 1:] = tm.astype(np.int64).cumsum(1).cumsum(2)

    tot_sum = 0.0
    tot_cnt = 0
    for b in range(B):
        cm = coarse[b] > 0.5
        if not cm.any():
            continue
        crmin, crmax, ccmin, ccmax = _cc_boxes(cm)
        r0 = np.maximum(BLK * crmin - DIL, 0)
        r1 = np.minimum(BLK * (crmax + 1) + DIL, H)
        c0 = np.maximum(BLK * ccmin - DIL, 0)
        c1 = np.minimum(BLK * (ccmax + 1) + DIL, W)

        core, slot = b // BPC, b % BPC
        s_pm = res.results[core]["sat_pm"][slot].astype(np.float64)
        s_p = res.results[core]["sat_p"][slot].astype(np.float64)
        s_pm[:, 0] = 0.0
        s_pm[0, :] = 0.0
        s_p[:, 0] = 0.0
        s_p[0, :] = 0.0

        i0 = _lo_index(r0)
        i1 = _hi_index(r1)
        j0 = _lo_index(c0)
        j1 = _hi_index(c1)

        def box(s):
            return s[i1, j1] - s[i0, j1] - s[i1, j0] + s[i0, j0]

        inter = box(s_pm)
        psum = box(s_p)
        msum = (
            sat_m[b][r1, c1] - sat_m[b][r0, c1]
            - sat_m[b][r1, c0] + sat_m[b][r0, c0]
        ).astype(np.float64)
        dice = (2.0 * inter + 1.0) / (psum + msum + 1.0)
        tot_sum += np.sum(1.0 - dice)
        tot_cnt += len(dice)

    l_comp = tot_sum / max(tot_cnt, 1) if tot_cnt > 0 else 0.0
    return np.array(bce + l_comp, dtype=np.float32)


# revision 3
# speedup vs baseline: 16.1509x; 16.1509x over previous
"""Trainium2 Bass kernel for nn_ComponentIoULoss.

Loss = global BCE (label smoothing, pos_weight=1) + mean per-connected-component
crop-dice over dilated component bboxes.

Device strategy (8 NeuronCores, data-parallel over batch, 2 images/core):
  - BCE partial sums on-device: sum(w*x) with w = 1 - smoothed(t), and
    sum(ln sigmoid(x))  (softplus(-x) = -ln sigmoid(x);
    BCE elem = (1-t')x + softplus(-x)).
  - Per-component crop dice needs box sums of p=sigmoid(x) and p*m over
    dilated component bboxes.  Targets are 8x8-blocky, so dilated bbox
    corners can only take 128 distinct values per axis:
    {0} + {8k-3 : k=1..63} + {8m+3 : m=1..63} + {512}.
    The device computes a "selected summed-area table": T1 = Lsel^T @ A
    (Lsel [512 x 128] prefix-indicator matrix, A in {p*m, p}), a cumulative
    sum scan along the free axis, packs the 128 selected columns on-chip and
    DMAs one contiguous SAT[128 x 128] per image per matrix.  The host reads
    4 SAT corners per component (exact prefix-sum differences).
  - Connected-component labeling (tiny integer problem, 64x64 coarse grid
    per image) and the final scalar combine run on the host.
"""

import sys

if "/opt/trn_rl_repo" not in sys.path:
    sys.path.insert(0, "/opt/trn_rl_repo")

import numpy as np
import ml_dtypes

N_CORES = 8
B, H, W = 16, 512, 512
BPC = B // N_CORES  # batches per core
BLK = 8
NC_COARSE = H // BLK  # 64
DIL = 3
EPS = 0.05
NSEL = 128

_COMPILED = {}


# ---------------------------------------------------------------------------
# selector geometry
# ---------------------------------------------------------------------------

def _sel_values():
    """The 128 prefix lengths v such that SAT[q] = sum over rows i < v[q]."""
    vs = [0]
    vs += [8 * k - 3 for k in range(1, 64)]
    vs += [8 * m + 3 for m in range(1, 64)]
    vs += [512]
    return np.array(vs, dtype=np.int64)


def _lo_index(r0):
    """Selector index for a box low edge r0 in {0} | {8k-3}."""
    return np.where(r0 == 0, 0, (r0 + 3) // 8)


def _hi_index(r1):
    """Selector index for a box high edge r1 in {8m+3} | {512}."""
    return np.where(r1 == 512, 127, 63 + (r1 - 3) // 8)


def _lsel_host():
    """Lsel^T [512 x 128] bf16 as [4, 128, 128] contract-chunk-major."""
    vs = _sel_values()
    i = np.arange(512)
    m = (i[:, None] < vs[None, :]).astype(ml_dtypes.bfloat16)
    return np.ascontiguousarray(m.reshape(4, 128, 128))


# ---------------------------------------------------------------------------
# host connected components (min-index propagation, mirrors reference)
# ---------------------------------------------------------------------------

def _cc_boxes(mask):
    """4-connected components of bool [h, w]. Returns (rmin, rmax, cmin, cmax)
    int arrays, one entry per component."""
    h, w = mask.shape
    n = h * w
    idx = np.arange(n, dtype=np.int64).reshape(h, w)
    lab = np.where(mask, idx, n)
    big = n
    while True:
        p = np.pad(lab, 1, constant_values=big)
        nmin = np.minimum(
            np.minimum(p[:-2, 1:-1], p[2:, 1:-1]),
            np.minimum(p[1:-1, :-2], p[1:-1, 2:]),
        )
        l2 = np.where(mask, np.minimum(lab, nmin), big)
        flat = np.append(l2.ravel(), big)
        flat = flat[flat]
        flat = flat[flat]
        l2 = flat[:-1].reshape(h, w)
        if np.array_equal(l2, lab):
            break
        lab = l2
    rows, cols = np.nonzero(mask)
    labs = lab[mask]
    uniq, inv = np.unique(labs, return_inverse=True)
    k = len(uniq)
    rmin = np.full(k, h, dtype=np.int64)
    rmax = np.full(k, -1, dtype=np.int64)
    cmin = np.full(k, w, dtype=np.int64)
    cmax = np.full(k, -1, dtype=np.int64)
    np.minimum.at(rmin, inv, rows)
    np.maximum.at(rmax, inv, rows)
    np.minimum.at(cmin, inv, cols)
    np.maximum.at(cmax, inv, cols)
    return rmin, rmax, cmin, cmax


# ---------------------------------------------------------------------------
# device kernel
# ---------------------------------------------------------------------------

def _build():
    import concourse.bacc as bacc
    import concourse.tile as tile
    from concourse import mybir
    from concourse.bass import _add_dep_helper

    F32 = mybir.dt.float32
    BF16 = mybir.dt.bfloat16
    AF = mybir.ActivationFunctionType
    OP = mybir.AluOpType

    nc = bacc.Bacc("TRN2", target_bir_lowering=False, debug=False,
                   num_devices=N_CORES)

    logit_d = nc.dram_tensor("logit", [BPC, H, W], F32, kind="ExternalInput")
    tbf_d = nc.dram_tensor("tbf", [BPC, H, W], BF16, kind="ExternalInput")
    wbf_d = nc.dram_tensor("wbf", [BPC, H, W], BF16, kind="ExternalInput")
    lsel_d = nc.dram_tensor("lsel", [4, 128, 128], BF16, kind="ExternalInput")
    sat_pm_d = nc.dram_tensor("sat_pm", [BPC, 128, 128], F32, kind="ExternalOutput")
    sat_p_d = nc.dram_tensor("sat_p", [BPC, 128, 128], F32, kind="ExternalOutput")
    accs_d = nc.dram_tensor("accs", [2 * BPC, 128, 1], F32, kind="ExternalOutput")

    with tile.TileContext(nc) as tc:
        with (
            tc.tile_pool(name="io", bufs=1) as io_pool,
            tc.tile_pool(name="mid", bufs=1) as mid_pool,
            tc.tile_pool(name="scr", bufs=1) as scr_pool,
            tc.tile_pool(name="scan", bufs=1) as scan_pool,
            tc.tile_pool(name="small", bufs=1) as small_pool,
            tc.tile_pool(name="ps", bufs=1, space="PSUM") as ps_pool,
        ):
            lsel = small_pool.tile([128, 4, 128], BF16, tag="lsel")
            nc.sync.dma_start(
                lsel[:], lsel_d.ap().rearrange("c p q -> p c q")
            )
            dummy = small_pool.tile([128, 512], F32, tag="dummy")
            nc.gpsimd.memset(dummy[:], 0.0)

            xs, tbs, wbs = [], [], []
            for b in range(BPC):
                x = io_pool.tile([128, 2048], F32, tag=f"x{b}")
                nc.sync.dma_start(
                    x[:].rearrange("p (c w) -> p c w", w=W),
                    logit_d.ap()[b].rearrange("(c p) w -> p c w", p=128),
                )
                tb = io_pool.tile([128, 2048], BF16, tag=f"t{b}")
                nc.sync.dma_start(
                    tb[:].rearrange("p (c w) -> p c w", w=W),
                    tbf_d.ap()[b].rearrange("(c p) w -> p c w", p=128),
                )
                wb = io_pool.tile([128, 2048], BF16, tag=f"w{b}")
                nc.sync.dma_start(
                    wb[:].rearrange("p (c w) -> p c w", w=W),
                    wbf_d.ap()[b].rearrange("(c p) w -> p c w", p=128),
                )
                xs.append(x)
                tbs.append(tb)
                wbs.append(wb)

            p_bfs, sig_insts = [], []
            for b in range(BPC):
                p_bf = mid_pool.tile([128, 2048], BF16, tag=f"p{b}")
                sig_insts.append(
                    nc.scalar.activation(p_bf[:], xs[b][:], AF.Sigmoid)
                )
                p_bfs.append(p_bf)

            for b in range(BPC):
                x, tb, wb, p_bf = xs[b], tbs[b], wbs[b], p_bfs[b]

                # --- BCE partial sums ---
                acc_ln = small_pool.tile([128, 1], F32, tag=f"acc_ln{b}")
                lnscr = scr_pool.tile([128, 2048], BF16, tag=f"lnscr{b}")
                ln_inst = nc.scalar.activation(
                    lnscr[:], p_bf[:], AF.Ln, accum_out=acc_ln[:]
                )
                # keep ACT ordered sigmoid,sigmoid,ln,ln -> 2 act-table loads
                _add_dep_helper(ln_inst.ins, sig_insts[-1].ins, sync=False,
                                reason="group act tables")
                nc.sync.dma_start(accs_d.ap()[b], acc_ln[:])

                # wx = w * x on GpSimd (otherwise idle)
                wx = scr_pool.tile([128, 2048], BF16, tag=f"wx{b}")
                nc.gpsimd.tensor_tensor(wx[:], wb[:], x[:], OP.mult)
                acc_wx = small_pool.tile([128, 1], F32, tag=f"acc_wx{b}")
                wxscr = scr_pool.tile([128, 2048], BF16, tag=f"wxscr{b}")
                nc.vector.tensor_scalar(
                    wxscr[:], wx[:], 1.0, None, OP.mult, OP.add,
                    accum_out=acc_wx[:],
                )
                nc.sync.dma_start(accs_d.ap()[BPC + b], acc_wx[:])

                # --- a1 = p * m (exact: m is 0/1, both bf16) ---
                a1 = mid_pool.tile([128, 2048], BF16, tag=f"a1{b}")
                nc.vector.tensor_tensor(a1[:], p_bf[:], tb[:], OP.mult)

                # --- selected SAT for each matrix ---
                for name, rhs, out_d in (
                    ("pm", a1, sat_pm_d),
                    ("p", p_bf, sat_p_d),
                ):
                    t1 = ps_pool.tile([128, 512], F32, tag=f"t1{name}{b}")
                    rhs3 = rhs[:].rearrange("p (c w) -> p c w", w=W)
                    for c in range(4):
                        nc.tensor.matmul(
                            t1[:], lsel[:, c, :], rhs3[:, c, :],
                            start=(c == 0), stop=(c == 3),
                        )
                    scan = scan_pool.tile([128, 512], F32, tag=f"scan{name}{b}")
                    nc.vector.tensor_tensor_scan(
                        scan[:], t1[:], dummy[:], 0.0, OP.add, OP.add
                    )
                    # pack the 128 selected columns (col 0 = empty prefix is
                    # zeroed host-side) and ship one contiguous block
                    pk = scan_pool.tile([128, 128], F32, tag=f"pk{name}{b}")
                    nc.vector.tensor_copy(pk[:, 1:64], scan[:, 4:508:8])
                    nc.vector.tensor_copy(pk[:, 64:127], scan[:, 10:512:8])
                    nc.vector.tensor_copy(pk[:, 127:128], scan[:, 511:512])
                    nc.sync.dma_start(out_d.ap()[b], pk[:])

    nc.compile()
    return nc


def _get_compiled():
    if "nc" not in _COMPILED:
        _COMPILED["nc"] = _build()
    return _COMPILED["nc"]


def _in_maps(logit, target):
    """Shard full [B,1,H,W] f32 inputs into per-core input maps."""
    lsel = _lsel_host()
    tbf = target[:, 0].astype(ml_dtypes.bfloat16)
    wbf = ((1.0 - EPS) - (1.0 - 2.0 * EPS) * target[:, 0]).astype(
        ml_dtypes.bfloat16
    )
    return [
        {
            "logit": np.ascontiguousarray(logit[c * BPC:(c + 1) * BPC, 0]),
            "tbf": np.ascontiguousarray(tbf[c * BPC:(c + 1) * BPC]),
            "wbf": np.ascontiguousarray(wbf[c * BPC:(c + 1) * BPC]),
            "lsel": lsel,
        }
        for c in range(N_CORES)
    ]


# ---------------------------------------------------------------------------
# host fallback (general inputs; not hit for blocky targets)
# ---------------------------------------------------------------------------

def _host_reference(logit, target):
    x = logit[:, 0].astype(np.float64)
    t = target[:, 0].astype(np.float64)
    ts = t * (1.0 - EPS) + (1.0 - t) * EPS
    bce = np.mean((1.0 - ts) * x + np.logaddexp(0.0, -x))
    p = 1.0 / (1.0 + np.exp(-x))
    tot_sum = 0.0
    tot_cnt = 0
    for b in range(B):
        mask = t[b] > 0.5
        if not mask.any():
            continue
        rmin, rmax, cmin, cmax = _cc_boxes(mask)
        r0 = np.clip(rmin - DIL, 0, H)
        r1 = np.clip(rmax + 1 + DIL, 0, H)
        c0 = np.clip(cmin - DIL, 0, W)
        c1 = np.clip(cmax + 1 + DIL, 0, W)
        sat_pm = np.zeros((H + 1, W + 1))
        sat_pm[1:, 1:] = (p[b] * t[b]).cumsum(0).cumsum(1)
        sat_p = np.zeros((H + 1, W + 1))
        sat_p[1:, 1:] = p[b].cumsum(0).cumsum(1)
        sat_m = np.zeros((H + 1, W + 1))
        sat_m[1:, 1:] = t[b].cumsum(0).cumsum(1)

        def box(s):
            return s[r1, c1] - s[r0, c1] - s[r1, c0] + s[r0, c0]

        dice = (2.0 * box(sat_pm) + 1.0) / (box(sat_p) + box(sat_m) + 1.0)
        tot_sum += np.sum(1.0 - dice)
        tot_cnt += len(dice)
    l_comp = tot_sum / max(tot_cnt, 1) if tot_cnt > 0 else 0.0
    return np.array(bce + l_comp, dtype=np.float32)


# ---------------------------------------------------------------------------
# entry point
# ---------------------------------------------------------------------------

def kernel(logit, target):
    from concourse.bass_utils import run_bass_kernel_spmd

    logit = np.ascontiguousarray(np.asarray(logit, dtype=np.float32))
    target = np.ascontiguousarray(np.asarray(target, dtype=np.float32))
    assert logit.shape == (B, 1, H, W) and target.shape == (B, 1, H, W)

    tgt = target[:, 0]
    coarse = tgt[:, ::BLK, ::BLK]
    blocky = np.array_equal(
        tgt.reshape(B, NC_COARSE, BLK, NC_COARSE, BLK),
        np.broadcast_to(
            coarse[:, :, None, :, None],
            (B, NC_COARSE, BLK, NC_COARSE, BLK),
        ),
    )
    if not blocky:
        return _host_reference(logit, target)

    nc = _get_compiled()
    res = run_bass_kernel_spmd(
        nc, _in_maps(logit, target), core_ids=list(range(N_CORES))
    )

    # ---- BCE ----
    s_ln = 0.0
    s_wx = 0.0
    for c in range(N_CORES):
        accs = res.results[c]["accs"].astype(np.float64)
        s_ln += accs[0:BPC].sum()
        s_wx += accs[BPC:2 * BPC].sum()
    n_tot = float(B * H * W)
    # sum softplus(-x) = -sum ln p ; bce elem = w*x + softplus(-x)
    bce = (s_wx - s_ln) / n_tot

    # ---- per-component crop dice ----
    tm = tgt > 0.5
    sat_m = np.zeros((B, H + 1, W + 1), dtype=np.int64)
    sat_m[:, 1:, 1:] = tm.astype(np.int64).cumsum(1).cumsum(2)

    tot_sum = 0.0
    tot_cnt = 0
    for b in range(B):
        cm = coarse[b] > 0.5
        if not cm.any():
            continue
        crmin, crmax, ccmin, ccmax = _cc_boxes(cm)
        r0 = np.maximum(BLK * crmin - DIL, 0)
        r1 = np.minimum(BLK * (crmax + 1) + DIL, H)
        c0 = np.maximum(BLK * ccmin - DIL, 0)
        c1 = np.minimum(BLK * (ccmax + 1) + DIL, W)

        core, slot = b // BPC, b % BPC
        s_pm = res.results[core]["sat_pm"][slot].astype(np.float64)
        s_p = res.results[core]["sat_p"][slot].astype(np.float64)
        s_pm[:, 0] = 0.0
        s_pm[0, :] = 0.0
        s_p[:, 0] = 0.0
        s_p[0, :] = 0.0

        i0 = _lo_index(r0)
        i1 = _hi_index(r1)
        j0 = _lo_index(c0)
        j1 = _hi_index(c1)

        def box(s):
            return s[i1, j1] - s[i0, j1] - s[i1, j0] + s[i0, j0]

        inter = box(s_pm)
        psum = box(s_p)
        msum = (
            sat_m[b][r1, c1] - sat_m[b][r0, c1]
            - sat_m[b][r1, c0] + sat_m[b][r0, c0]
        ).astype(np.float64)
        dice = (2.0 * inter + 1.0) / (psum + msum + 1.0)
        tot_sum += np.sum(1.0 - dice)
        tot_cnt += len(dice)

    l_comp = tot_sum / max(tot_cnt, 1) if tot_cnt > 0 else 0.0
    return np.array(bce + l_comp, dtype=np.float32)


# revision 9
# speedup vs baseline: 17.9722x; 1.1128x over previous
"""Trainium2 Bass kernel for nn_ComponentIoULoss.

Loss = global BCE (label smoothing, pos_weight=1) + mean per-connected-component
crop-dice over dilated component bboxes.

Device strategy (8 NeuronCores, data-parallel over batch, 2 images/core):
  - BCE partial sums on-device: sum(w*x) with w = 1 - smoothed(t), and
    sum(ln sigmoid(x))  (softplus(-x) = -ln sigmoid(x);
    BCE elem = (1-t')x + softplus(-x)).
  - Per-component crop dice needs box sums of p=sigmoid(x) and p*m over
    dilated component bboxes.  Targets are 8x8-blocky, so dilated bbox
    corners can only take 128 distinct values per axis:
    {0} + {8k-3 : k=1..63} + {8m+3 : m=1..63} + {512}.
    The device computes a "selected summed-area table": T1 = Lsel^T @ A
    (Lsel [512 x 128] prefix-indicator matrix, A in {p*m, p}), a cumulative
    sum scan along the free axis, packs the 128 selected columns on-chip and
    DMAs one contiguous SAT[128 x 128] per image per matrix.  The host reads
    4 SAT corners per component (exact prefix-sum differences).
  - Connected-component labeling (tiny integer problem, 64x64 coarse grid
    per image) and the final scalar combine run on the host.
"""

import sys

if "/opt/trn_rl_repo" not in sys.path:
    sys.path.insert(0, "/opt/trn_rl_repo")

import numpy as np
import ml_dtypes

N_CORES = 8
B, H, W = 16, 512, 512
BPC = B // N_CORES  # batches per core
BLK = 8
NC_COARSE = H // BLK  # 64
DIL = 3
EPS = 0.05
NSEL = 128

_COMPILED = {}


# ---------------------------------------------------------------------------
# selector geometry
# ---------------------------------------------------------------------------

def _sel_values():
    """The 128 prefix lengths v such that SAT[q] = sum over rows i < v[q]."""
    vs = [0]
    vs += [8 * k - 3 for k in range(1, 64)]
    vs += [8 * m + 3 for m in range(1, 64)]
    vs += [512]
    return np.array(vs, dtype=np.int64)


def _lo_index(r0):
    """Selector index for a box low edge r0 in {0} | {8k-3}."""
    return np.where(r0 == 0, 0, (r0 + 3) // 8)


def _hi_index(r1):
    """Selector index for a box high edge r1 in {8m+3} | {512}."""
    return np.where(r1 == 512, 127, 63 + (r1 - 3) // 8)


def _lsel_host():
    """Lsel^T [512 x 128] bf16 as [4, 128, 128] contract-chunk-major."""
    vs = _sel_values()
    i = np.arange(512)
    m = (i[:, None] < vs[None, :]).astype(ml_dtypes.bfloat16)
    return np.ascontiguousarray(m.reshape(4, 128, 128))


# ---------------------------------------------------------------------------
# host connected components (min-index propagation, mirrors reference)
# ---------------------------------------------------------------------------

def _cc_boxes(mask):
    """4-connected components of bool [h, w]. Returns (rmin, rmax, cmin, cmax)
    int arrays, one entry per component."""
    h, w = mask.shape
    n = h * w
    idx = np.arange(n, dtype=np.int64).reshape(h, w)
    lab = np.where(mask, idx, n)
    big = n
    while True:
        p = np.pad(lab, 1, constant_values=big)
        nmin = np.minimum(
            np.minimum(p[:-2, 1:-1], p[2:, 1:-1]),
            np.minimum(p[1:-1, :-2], p[1:-1, 2:]),
        )
        l2 = np.where(mask, np.minimum(lab, nmin), big)
        flat = np.append(l2.ravel(), big)
        flat = flat[flat]
        flat = flat[flat]
        l2 = flat[:-1].reshape(h, w)
        if np.array_equal(l2, lab):
            break
        lab = l2
    rows, cols = np.nonzero(mask)
    labs = lab[mask]
    uniq, inv = np.unique(labs, return_inverse=True)
    k = len(uniq)
    rmin = np.full(k, h, dtype=np.int64)
    rmax = np.full(k, -1, dtype=np.int64)
    cmin = np.full(k, w, dtype=np.int64)
    cmax = np.full(k, -1, dtype=np.int64)
    np.minimum.at(rmin, inv, rows)
    np.maximum.at(rmax, inv, rows)
    np.minimum.at(cmin, inv, cols)
    np.maximum.at(cmax, inv, cols)
    return rmin, rmax, cmin, cmax


# ---------------------------------------------------------------------------
# device kernel
# ---------------------------------------------------------------------------

def _build():
    import concourse.bacc as bacc
    import concourse.tile as tile
    from concourse import mybir
    from concourse.bass import _add_dep_helper

    F32 = mybir.dt.float32
    BF16 = mybir.dt.bfloat16
    AF = mybir.ActivationFunctionType
    OP = mybir.AluOpType

    nc = bacc.Bacc("TRN2", target_bir_lowering=False, debug=False,
                   num_devices=N_CORES)

    logit_d = nc.dram_tensor("logit", [BPC, H, W], F32, kind="ExternalInput")
    tbf_d = nc.dram_tensor("tbf", [BPC, H, W], BF16, kind="ExternalInput")
    wbf_d = nc.dram_tensor("wbf", [BPC, H, W], BF16, kind="ExternalInput")
    lsel_d = nc.dram_tensor("lsel", [4, 128, 128], BF16, kind="ExternalInput")
    # single packed output: [sat_pm b0 | sat_p b0 | sat_pm b1 | sat_p b1 |
    #                        acc_ln b0 | acc_wx b0 | acc_ln b1 | acc_wx b1]
    out_d = nc.dram_tensor("out_all", [128, 2 * 128 * BPC + 2 * BPC], F32,
                           kind="ExternalOutput")

    with tile.TileContext(nc) as tc:
        with (
            tc.tile_pool(name="io", bufs=1) as io_pool,
            tc.tile_pool(name="mid", bufs=1) as mid_pool,
            tc.tile_pool(name="scr", bufs=1) as scr_pool,
            tc.tile_pool(name="scan", bufs=1) as scan_pool,
            tc.tile_pool(name="small", bufs=1) as small_pool,
            tc.tile_pool(name="ps", bufs=1, space="PSUM") as ps_pool,
        ):
            dummy = small_pool.tile([128, 512], F32, tag="dummy")
            nc.gpsimd.memset(dummy[:], 0.0)
            outt = small_pool.tile([128, 2 * 128 * BPC + 2 * BPC], F32,
                                   tag="outt")

            xs, tbs, wbs = [], [], []
            for b in range(BPC):
                x = io_pool.tile([128, 2048], F32, tag=f"x{b}")
                nc.sync.dma_start(
                    x[:].rearrange("p (c w) -> p c w", w=W),
                    logit_d.ap()[b].rearrange("(c p) w -> p c w", p=128),
                )
                tb = io_pool.tile([128, 2048], BF16, tag=f"t{b}")
                nc.sync.dma_start(
                    tb[:].rearrange("p (c w) -> p c w", w=W),
                    tbf_d.ap()[b].rearrange("(c p) w -> p c w", p=128),
                )
                wb = io_pool.tile([128, 2048], BF16, tag=f"w{b}")
                nc.sync.dma_start(
                    wb[:].rearrange("p (c w) -> p c w", w=W),
                    wbf_d.ap()[b].rearrange("(c p) w -> p c w", p=128),
                )
                xs.append(x)
                tbs.append(tb)
                wbs.append(wb)

            lsel = small_pool.tile([128, 4, 128], BF16, tag="lsel")
            nc.sync.dma_start(
                lsel[:], lsel_d.ap().rearrange("c p q -> p c q")
            )

            p_bfs, sig_insts = [], []
            for b in range(BPC):
                p_bf = mid_pool.tile([128, 2048], BF16, tag=f"p{b}")
                sig_insts.append(
                    nc.scalar.activation(p_bf[:], xs[b][:], AF.Sigmoid)
                )
                p_bfs.append(p_bf)

            for b in range(BPC):
                x, tb, wb, p_bf = xs[b], tbs[b], wbs[b], p_bfs[b]

                # --- BCE partial sums ---
                acc_ln = small_pool.tile([128, 1], F32, tag=f"acc_ln{b}")
                lnscr = scr_pool.tile([128, 2048], BF16, tag=f"lnscr{b}")
                ln_inst = nc.scalar.activation(
                    lnscr[:], p_bf[:], AF.Ln, accum_out=acc_ln[:]
                )
                # keep ACT ordered sigmoid,sigmoid,ln,ln -> 2 act-table loads
                _add_dep_helper(ln_inst.ins, sig_insts[-1].ins, sync=False,
                                reason="group act tables")
                nc.vector.tensor_copy(outt[:, 512 + 2 * b:513 + 2 * b],
                                      acc_ln[:])

                # wx = w * x on GpSimd (otherwise idle)
                wx = scr_pool.tile([128, 2048], BF16, tag=f"wx{b}")
                nc.gpsimd.tensor_tensor(wx[:], wb[:], x[:], OP.mult)
                acc_wx = small_pool.tile([128, 1], F32, tag=f"acc_wx{b}")
                wxscr = scr_pool.tile([128, 2048], BF16, tag=f"wxscr{b}")
                nc.vector.tensor_scalar(
                    wxscr[:], wx[:], 1.0, None, OP.mult, OP.add,
                    accum_out=acc_wx[:],
                )
                nc.vector.tensor_copy(outt[:, 513 + 2 * b:514 + 2 * b],
                                      acc_wx[:])

                # --- a1 = p * m (exact: m is 0/1, both bf16) ---
                a1 = mid_pool.tile([128, 2048], BF16, tag=f"a1{b}")
                nc.vector.tensor_tensor(a1[:], p_bf[:], tb[:], OP.mult)

                # --- selected SAT for each matrix ---
                for mi, (name, rhs) in enumerate((("pm", a1), ("p", p_bf))):
                    t1 = ps_pool.tile([128, 512], F32, tag=f"t1{name}{b}")
                    rhs3 = rhs[:].rearrange("p (c w) -> p c w", w=W)
                    for c in range(4):
                        nc.tensor.matmul(
                            t1[:], lsel[:, c, :], rhs3[:, c, :],
                            start=(c == 0), stop=(c == 3),
                        )
                    scan = scan_pool.tile([128, 512], F32, tag=f"scan{name}{b}")
                    nc.vector.tensor_tensor_scan(
                        scan[:], t1[:], dummy[:], 0.0, OP.add, OP.add
                    )
                    # pack the 128 selected columns (col 0 = empty prefix is
                    # zeroed host-side) into the combined output tile
                    base = (2 * b + mi) * 128
                    nc.vector.tensor_copy(outt[:, base + 1:base + 64],
                                          scan[:, 4:508:8])
                    nc.vector.tensor_copy(outt[:, base + 64:base + 127],
                                          scan[:, 10:512:8])
                    nc.vector.tensor_copy(outt[:, base + 127:base + 128],
                                          scan[:, 511:512])

            nc.sync.dma_start(out_d.ap(), outt[:])

    nc.compile()
    return nc


def _get_compiled():
    if "nc" not in _COMPILED:
        _COMPILED["nc"] = _build()
    return _COMPILED["nc"]


def _in_maps(logit, target):
    """Shard full [B,1,H,W] f32 inputs into per-core input maps."""
    lsel = _lsel_host()
    tbf = target[:, 0].astype(ml_dtypes.bfloat16)
    wbf = ((1.0 - EPS) - (1.0 - 2.0 * EPS) * target[:, 0]).astype(
        ml_dtypes.bfloat16
    )
    return [
        {
            "logit": np.ascontiguousarray(logit[c * BPC:(c + 1) * BPC, 0]),
            "tbf": np.ascontiguousarray(tbf[c * BPC:(c + 1) * BPC]),
            "wbf": np.ascontiguousarray(wbf[c * BPC:(c + 1) * BPC]),
            "lsel": lsel,
        }
        for c in range(N_CORES)
    ]


# ---------------------------------------------------------------------------
# host fallback (general inputs; not hit for blocky targets)
# ---------------------------------------------------------------------------

def _host_reference(logit, target):
    x = logit[:, 0].astype(np.float64)
    t = target[:, 0].astype(np.float64)
    ts = t * (1.0 - EPS) + (1.0 - t) * EPS
    bce = np.mean((1.0 - ts) * x + np.logaddexp(0.0, -x))
    p = 1.0 / (1.0 + np.exp(-x))
    tot_sum = 0.0
    tot_cnt = 0
    for b in range(B):
        mask = t[b] > 0.5
        if not mask.any():
            continue
        rmin, rmax, cmin, cmax = _cc_boxes(mask)
        r0 = np.clip(rmin - DIL, 0, H)
        r1 = np.clip(rmax + 1 + DIL, 0, H)
        c0 = np.clip(cmin - DIL, 0, W)
        c1 = np.clip(cmax + 1 + DIL, 0, W)
        sat_pm = np.zeros((H + 1, W + 1))
        sat_pm[1:, 1:] = (p[b] * t[b]).cumsum(0).cumsum(1)
        sat_p = np.zeros((H + 1, W + 1))
        sat_p[1:, 1:] = p[b].cumsum(0).cumsum(1)
        sat_m = np.zeros((H + 1, W + 1))
        sat_m[1:, 1:] = t[b].cumsum(0).cumsum(1)

        def box(s):
            return s[r1, c1] - s[r0, c1] - s[r1, c0] + s[r0, c0]

        dice = (2.0 * box(sat_pm) + 1.0) / (box(sat_p) + box(sat_m) + 1.0)
        tot_sum += np.sum(1.0 - dice)
        tot_cnt += len(dice)
    l_comp = tot_sum / max(tot_cnt, 1) if tot_cnt > 0 else 0.0
    return np.array(bce + l_comp, dtype=np.float32)


# ---------------------------------------------------------------------------
# entry point
# ---------------------------------------------------------------------------

def kernel(logit, target):
    from concourse.bass_utils import run_bass_kernel_spmd

    logit = np.ascontiguousarray(np.asarray(logit, dtype=np.float32))
    target = np.ascontiguousarray(np.asarray(target, dtype=np.float32))
    assert logit.shape == (B, 1, H, W) and target.shape == (B, 1, H, W)

    tgt = target[:, 0]
    coarse = tgt[:, ::BLK, ::BLK]
    blocky = np.array_equal(
        tgt.reshape(B, NC_COARSE, BLK, NC_COARSE, BLK),
        np.broadcast_to(
            coarse[:, :, None, :, None],
            (B, NC_COARSE, BLK, NC_COARSE, BLK),
        ),
    )
    if not blocky:
        return _host_reference(logit, target)

    nc = _get_compiled()
    res = run_bass_kernel_spmd(
        nc, _in_maps(logit, target), core_ids=list(range(N_CORES))
    )

    # ---- BCE ----
    s_ln = 0.0
    s_wx = 0.0
    for c in range(N_CORES):
        out = res.results[c]["out_all"].astype(np.float64)
        for b in range(BPC):
            s_ln += out[:, 512 + 2 * b].sum()
            s_wx += out[:, 513 + 2 * b].sum()
    n_tot = float(B * H * W)
    # sum softplus(-x) = -sum ln p ; bce elem = w*x + softplus(-x)
    bce = (s_wx - s_ln) / n_tot

    # ---- per-component crop dice ----
    tm = tgt > 0.5
    sat_m = np.zeros((B, H + 1, W + 1), dtype=np.int64)
    sat_m[:, 1:, 1:] = tm.astype(np.int64).cumsum(1).cumsum(2)

    tot_sum = 0.0
    tot_cnt = 0
    for b in range(B):
        cm = coarse[b] > 0.5
        if not cm.any():
            continue
        crmin, crmax, ccmin, ccmax = _cc_boxes(cm)
        r0 = np.maximum(BLK * crmin - DIL, 0)
        r1 = np.minimum(BLK * (crmax + 1) + DIL, H)
        c0 = np.maximum(BLK * ccmin - DIL, 0)
        c1 = np.minimum(BLK * (ccmax + 1) + DIL, W)

        core, slot = b // BPC, b % BPC
        out = res.results[core]["out_all"]
        s_pm = out[:, (2 * slot) * 128:(2 * slot) * 128 + 128].astype(np.float64)
        s_p = out[:, (2 * slot + 1) * 128:(2 * slot + 1) * 128 + 128].astype(np.float64)
        s_pm[:, 0] = 0.0
        s_pm[0, :] = 0.0
        s_p[:, 0] = 0.0
        s_p[0, :] = 0.0

        i0 = _lo_index(r0)
        i1 = _hi_index(r1)
        j0 = _lo_index(c0)
        j1 = _hi_index(c1)

        def box(s):
            return s[i1, j1] - s[i0, j1] - s[i1, j0] + s[i0, j0]

        inter = box(s_pm)
        psum = box(s_p)
        msum = (
            sat_m[b][r1, c1] - sat_m[b][r0, c1]
            - sat_m[b][r1, c0] + sat_m[b][r0, c0]
        ).astype(np.float64)
        dice = (2.0 * inter + 1.0) / (psum + msum + 1.0)
        tot_sum += np.sum(1.0 - dice)
        tot_cnt += len(dice)

    l_comp = tot_sum / max(tot_cnt, 1) if tot_cnt > 0 else 0.0
    return np.array(bce + l_comp, dtype=np.float32)


# revision 12
# speedup vs baseline: 19.1088x; 1.0632x over previous
"""Trainium2 Bass kernel for nn_ComponentIoULoss.

Loss = global BCE (label smoothing, pos_weight=1) + mean per-connected-component
crop-dice over dilated component bboxes.

Device strategy (8 NeuronCores, data-parallel over batch, 2 images/core):
  - BCE partial sums on-device: sum(w*x) with w = 1 - smoothed(t), and
    sum(ln sigmoid(x))  (softplus(-x) = -ln sigmoid(x);
    BCE elem = (1-t')x + softplus(-x)).
  - Per-component crop dice needs box sums of p=sigmoid(x) and p*m over
    dilated component bboxes.  Targets are 8x8-blocky, so dilated bbox
    corners can only take 128 distinct values per axis:
    {0} + {8k-3 : k=1..63} + {8m+3 : m=1..63} + {512}.
    The device computes a "selected summed-area table": T1 = Lsel^T @ A
    (Lsel [512 x 128] prefix-indicator matrix, A in {p*m, p}), a cumulative
    sum scan along the free axis, packs the 128 selected columns on-chip and
    DMAs one contiguous SAT[128 x 128] per image per matrix.  The host reads
    4 SAT corners per component (exact prefix-sum differences).
  - Connected-component labeling (tiny integer problem, 64x64 coarse grid
    per image) and the final scalar combine run on the host.
"""

import sys

if "/opt/trn_rl_repo" not in sys.path:
    sys.path.insert(0, "/opt/trn_rl_repo")

import numpy as np
import ml_dtypes

N_CORES = 8
B, H, W = 16, 512, 512
BPC = B // N_CORES  # batches per core
BLK = 8
NC_COARSE = H // BLK  # 64
DIL = 3
EPS = 0.05
NSEL = 128

_COMPILED = {}


# ---------------------------------------------------------------------------
# selector geometry
# ---------------------------------------------------------------------------

def _sel_values():
    """The 128 prefix lengths v such that SAT[q] = sum over rows i < v[q]."""
    vs = [0]
    vs += [8 * k - 3 for k in range(1, 64)]
    vs += [8 * m + 3 for m in range(1, 64)]
    vs += [512]
    return np.array(vs, dtype=np.int64)


def _lo_index(r0):
    """Selector index for a box low edge r0 in {0} | {8k-3}."""
    return np.where(r0 == 0, 0, (r0 + 3) // 8)


def _hi_index(r1):
    """Selector index for a box high edge r1 in {8m+3} | {512}."""
    return np.where(r1 == 512, 127, 63 + (r1 - 3) // 8)


def _lsel_host():
    """Lsel^T [512 x 128] bf16 as [4, 128, 128] contract-chunk-major."""
    vs = _sel_values()
    i = np.arange(512)
    m = (i[:, None] < vs[None, :]).astype(ml_dtypes.bfloat16)
    return np.ascontiguousarray(m.reshape(4, 128, 128))


# ---------------------------------------------------------------------------
# host connected components (min-index propagation, mirrors reference)
# ---------------------------------------------------------------------------

def _cc_boxes(mask):
    """4-connected components of bool [h, w]. Returns (rmin, rmax, cmin, cmax)
    int arrays, one entry per component."""
    h, w = mask.shape
    n = h * w
    idx = np.arange(n, dtype=np.int64).reshape(h, w)
    lab = np.where(mask, idx, n)
    big = n
    while True:
        p = np.pad(lab, 1, constant_values=big)
        nmin = np.minimum(
            np.minimum(p[:-2, 1:-1], p[2:, 1:-1]),
            np.minimum(p[1:-1, :-2], p[1:-1, 2:]),
        )
        l2 = np.where(mask, np.minimum(lab, nmin), big)
        flat = np.append(l2.ravel(), big)
        flat = flat[flat]
        flat = flat[flat]
        l2 = flat[:-1].reshape(h, w)
        if np.array_equal(l2, lab):
            break
        lab = l2
    rows, cols = np.nonzero(mask)
    labs = lab[mask]
    uniq, inv = np.unique(labs, return_inverse=True)
    k = len(uniq)
    rmin = np.full(k, h, dtype=np.int64)
    rmax = np.full(k, -1, dtype=np.int64)
    cmin = np.full(k, w, dtype=np.int64)
    cmax = np.full(k, -1, dtype=np.int64)
    np.minimum.at(rmin, inv, rows)
    np.maximum.at(rmax, inv, rows)
    np.minimum.at(cmin, inv, cols)
    np.maximum.at(cmax, inv, cols)
    return rmin, rmax, cmin, cmax


# ---------------------------------------------------------------------------
# device kernel
# ---------------------------------------------------------------------------

def _build():
    import concourse.bacc as bacc
    import concourse.tile as tile
    from concourse import mybir
    from concourse.bass import _add_dep_helper

    F32 = mybir.dt.float32
    BF16 = mybir.dt.bfloat16
    AF = mybir.ActivationFunctionType
    OP = mybir.AluOpType

    nc = bacc.Bacc("TRN2", target_bir_lowering=False, debug=False,
                   num_devices=N_CORES)

    xbf_d = nc.dram_tensor("xbf", [BPC, H, W], BF16, kind="ExternalInput")
    tbf_d = nc.dram_tensor("tbf", [BPC, H, W], BF16, kind="ExternalInput")
    wbf_d = nc.dram_tensor("wbf", [BPC, H, W], BF16, kind="ExternalInput")
    lsel_d = nc.dram_tensor("lsel", [4, 128, 128], BF16, kind="ExternalInput")
    # single packed output: [sat_pm b0 | sat_p b0 | sat_pm b1 | sat_p b1 |
    #                        acc_ln b0 | acc_wx b0 | acc_ln b1 | acc_wx b1]
    out_d = nc.dram_tensor("out_all", [128, 2 * 128 * BPC + 2 * BPC], F32,
                           kind="ExternalOutput")

    with tile.TileContext(nc) as tc:
        with (
            tc.tile_pool(name="io", bufs=1) as io_pool,
            tc.tile_pool(name="mid", bufs=1) as mid_pool,
            tc.tile_pool(name="scr", bufs=1) as scr_pool,
            tc.tile_pool(name="scan", bufs=1) as scan_pool,
            tc.tile_pool(name="small", bufs=1) as small_pool,
            tc.tile_pool(name="ps", bufs=1, space="PSUM") as ps_pool,
        ):
            dummy = small_pool.tile([128, 512], F32, tag="dummy")
            nc.gpsimd.memset(dummy[:], 0.0)
            outt = small_pool.tile([128, 2 * 128 * BPC + 2 * BPC], F32,
                                   tag="outt")

            # x arrives in 4 row-chunks per image so sigmoid/a1/matmul start
            # as soon as the first 128 rows land
            xs, tbs, wbs = [], [], []
            for b in range(BPC):
                x = io_pool.tile([128, 2048], BF16, tag=f"x{b}")
                x3 = x[:].rearrange("p (c w) -> p c w", w=W)
                for c in range(4):
                    nc.sync.dma_start(
                        x3[:, c, :],
                        xbf_d.ap()[b, c * 128:(c + 1) * 128, :],
                    )
                tb = io_pool.tile([128, 2048], BF16, tag=f"t{b}")
                nc.sync.dma_start(
                    tb[:].rearrange("p (c w) -> p c w", w=W),
                    tbf_d.ap()[b].rearrange("(c p) w -> p c w", p=128),
                )
                xs.append(x)
                tbs.append(tb)
            lsel = small_pool.tile([128, 4, 128], BF16, tag="lsel")
            nc.sync.dma_start(
                lsel[:], lsel_d.ap().rearrange("c p q -> p c q")
            )
            for b in range(BPC):
                wb = io_pool.tile([128, 2048], BF16, tag=f"w{b}")
                nc.sync.dma_start(
                    wb[:].rearrange("p (c w) -> p c w", w=W),
                    wbf_d.ap()[b].rearrange("(c p) w -> p c w", p=128),
                )
                wbs.append(wb)

            # --- critical path: sigmoid -> a1 -> matmul -> scan -> pack ---
            p_bfs, a1s, sig_insts = [], [], []
            for b in range(BPC):
                x3 = xs[b][:].rearrange("p (c w) -> p c w", w=W)
                tb3 = tbs[b][:].rearrange("p (c w) -> p c w", w=W)
                p_bf = mid_pool.tile([128, 2048], BF16, tag=f"p{b}")
                p3 = p_bf[:].rearrange("p (c w) -> p c w", w=W)
                a1 = mid_pool.tile([128, 2048], BF16, tag=f"a1{b}")
                a13 = a1[:].rearrange("p (c w) -> p c w", w=W)
                t1_pm = ps_pool.tile([128, 512], F32, tag=f"t1pm{b}")
                t1_p = ps_pool.tile([128, 512], F32, tag=f"t1p{b}")
                for c in range(4):
                    sig_insts.append(
                        nc.scalar.activation(p3[:, c, :], x3[:, c, :],
                                             AF.Sigmoid)
                    )
                    nc.tensor.matmul(
                        t1_p[:], lsel[:, c, :], p3[:, c, :],
                        start=(c == 0), stop=(c == 3),
                    )
                    nc.vector.tensor_tensor(a13[:, c, :], p3[:, c, :],
                                            tb3[:, c, :], OP.mult)
                    nc.tensor.matmul(
                        t1_pm[:], lsel[:, c, :], a13[:, c, :],
                        start=(c == 0), stop=(c == 3),
                    )
                p_bfs.append(p_bf)
                a1s.append(a1)
                for mi, t1 in enumerate((t1_pm, t1_p)):
                    name = ("pm", "p")[mi]
                    scan = scan_pool.tile([128, 512], F32, tag=f"scan{name}{b}")
                    nc.vector.tensor_tensor_scan(
                        scan[:], t1[:], dummy[:], 0.0, OP.add, OP.add
                    )
                    # pack the 128 selected columns (col 0 = empty prefix is
                    # zeroed host-side) into the combined output tile
                    base = (2 * b + mi) * 128
                    nc.vector.tensor_copy(outt[:, base + 1:base + 64],
                                          scan[:, 4:508:8])
                    nc.vector.tensor_copy(outt[:, base + 64:base + 127],
                                          scan[:, 10:512:8])
                    nc.vector.tensor_copy(outt[:, base + 127:base + 128],
                                          scan[:, 511:512])

            # --- BCE partial sums (fill engine gaps; issued last) ---
            for b in range(BPC):
                acc_ln = small_pool.tile([128, 1], F32, tag=f"acc_ln{b}")
                lnscr = scr_pool.tile([128, 2048], BF16, tag=f"lnscr{b}")
                ln_inst = nc.scalar.activation(
                    lnscr[:], p_bfs[b][:], AF.Ln, accum_out=acc_ln[:]
                )
                # keep ACT ordered sigmoid*8, ln, ln -> 2 act-table loads
                _add_dep_helper(ln_inst.ins, sig_insts[-1].ins, sync=False,
                                reason="group act tables")
                nc.vector.tensor_copy(outt[:, 512 + 2 * b:513 + 2 * b],
                                      acc_ln[:])

                # wx = w * x on GpSimd (otherwise idle)
                wx = scr_pool.tile([128, 2048], BF16, tag=f"wx{b}")
                nc.gpsimd.tensor_tensor(wx[:], wbs[b][:], xs[b][:], OP.mult)
                acc_wx = small_pool.tile([128, 1], F32, tag=f"acc_wx{b}")
                wxscr = scr_pool.tile([128, 2048], BF16, tag=f"wxscr{b}")
                nc.vector.tensor_scalar(
                    wxscr[:], wx[:], 1.0, None, OP.mult, OP.add,
                    accum_out=acc_wx[:],
                )
                nc.vector.tensor_copy(outt[:, 513 + 2 * b:514 + 2 * b],
                                      acc_wx[:])

            nc.sync.dma_start(out_d.ap(), outt[:])

    nc.compile()
    return nc


def _get_compiled():
    if "nc" not in _COMPILED:
        _COMPILED["nc"] = _build()
    return _COMPILED["nc"]


def _in_maps(logit, target):
    """Shard full [B,1,H,W] f32 inputs into per-core input maps."""
    lsel = _lsel_host()
    xbf = logit[:, 0].astype(ml_dtypes.bfloat16)
    tbf = target[:, 0].astype(ml_dtypes.bfloat16)
    wbf = ((1.0 - EPS) - (1.0 - 2.0 * EPS) * target[:, 0]).astype(
        ml_dtypes.bfloat16
    )
    return [
        {
            "xbf": np.ascontiguousarray(xbf[c * BPC:(c + 1) * BPC]),
            "tbf": np.ascontiguousarray(tbf[c * BPC:(c + 1) * BPC]),
            "wbf": np.ascontiguousarray(wbf[c * BPC:(c + 1) * BPC]),
            "lsel": lsel,
        }
        for c in range(N_CORES)
    ]


# ---------------------------------------------------------------------------
# host fallback (general inputs; not hit for blocky targets)
# ---------------------------------------------------------------------------

def _host_reference(logit, target):
    x = logit[:, 0].astype(np.float64)
    t = target[:, 0].astype(np.float64)
    ts = t * (1.0 - EPS) + (1.0 - t) * EPS
    bce = np.mean((1.0 - ts) * x + np.logaddexp(0.0, -x))
    p = 1.0 / (1.0 + np.exp(-x))
    tot_sum = 0.0
    tot_cnt = 0
    for b in range(B):
        mask = t[b] > 0.5
        if not mask.any():
            continue
        rmin, rmax, cmin, cmax = _cc_boxes(mask)
        r0 = np.clip(rmin - DIL, 0, H)
        r1 = np.clip(rmax + 1 + DIL, 0, H)
        c0 = np.clip(cmin - DIL, 0, W)
        c1 = np.clip(cmax + 1 + DIL, 0, W)
        sat_pm = np.zeros((H + 1, W + 1))
        sat_pm[1:, 1:] = (p[b] * t[b]).cumsum(0).cumsum(1)
        sat_p = np.zeros((H + 1, W + 1))
        sat_p[1:, 1:] = p[b].cumsum(0).cumsum(1)
        sat_m = np.zeros((H + 1, W + 1))
        sat_m[1:, 1:] = t[b].cumsum(0).cumsum(1)

        def box(s):
            return s[r1, c1] - s[r0, c1] - s[r1, c0] + s[r0, c0]

        dice = (2.0 * box(sat_pm) + 1.0) / (box(sat_p) + box(sat_m) + 1.0)
        tot_sum += np.sum(1.0 - dice)
        tot_cnt += len(dice)
    l_comp = tot_sum / max(tot_cnt, 1) if tot_cnt > 0 else 0.0
    return np.array(bce + l_comp, dtype=np.float32)


# ---------------------------------------------------------------------------
# entry point
# ---------------------------------------------------------------------------

def kernel(logit, target):
    from concourse.bass_utils import run_bass_kernel_spmd

    logit = np.ascontiguousarray(np.asarray(logit, dtype=np.float32))
    target = np.ascontiguousarray(np.asarray(target, dtype=np.float32))
    assert logit.shape == (B, 1, H, W) and target.shape == (B, 1, H, W)

    tgt = target[:, 0]
    coarse = tgt[:, ::BLK, ::BLK]
    blocky = np.array_equal(
        tgt.reshape(B, NC_COARSE, BLK, NC_COARSE, BLK),
        np.broadcast_to(
            coarse[:, :, None, :, None],
            (B, NC_COARSE, BLK, NC_COARSE, BLK),
        ),
    )
    if not blocky:
        return _host_reference(logit, target)

    nc = _get_compiled()
    res = run_bass_kernel_spmd(
        nc, _in_maps(logit, target), core_ids=list(range(N_CORES))
    )

    # ---- BCE ----
    s_ln = 0.0
    s_wx = 0.0
    for c in range(N_CORES):
        out = res.results[c]["out_all"].astype(np.float64)
        for b in range(BPC):
            s_ln += out[:, 512 + 2 * b].sum()
            s_wx += out[:, 513 + 2 * b].sum()
    n_tot = float(B * H * W)
    # sum softplus(-x) = -sum ln p ; bce elem = w*x + softplus(-x)
    bce = (s_wx - s_ln) / n_tot

    # ---- per-component crop dice ----
    tm = tgt > 0.5
    sat_m = np.zeros((B, H + 1, W + 1), dtype=np.int64)
    sat_m[:, 1:, 1:] = tm.astype(np.int64).cumsum(1).cumsum(2)

    tot_sum = 0.0
    tot_cnt = 0
    for b in range(B):
        cm = coarse[b] > 0.5
        if not cm.any():
            continue
        crmin, crmax, ccmin, ccmax = _cc_boxes(cm)
        r0 = np.maximum(BLK * crmin - DIL, 0)
        r1 = np.minimum(BLK * (crmax + 1) + DIL, H)
        c0 = np.maximum(BLK * ccmin - DIL, 0)
        c1 = np.minimum(BLK * (ccmax + 1) + DIL, W)

        core, slot = b // BPC, b % BPC
        out = res.results[core]["out_all"]
        s_pm = out[:, (2 * slot) * 128:(2 * slot) * 128 + 128].astype(np.float64)
        s_p = out[:, (2 * slot + 1) * 128:(2 * slot + 1) * 128 + 128].astype(np.float64)
        s_pm[:, 0] = 0.0
        s_pm[0, :] = 0.0
        s_p[:, 0] = 0.0
        s_p[0, :] = 0.0

        i0 = _lo_index(r0)
        i1 = _hi_index(r1)
        j0 = _lo_index(c0)
        j1 = _hi_index(c1)

        def box(s):
            return s[i1, j1] - s[i0, j1] - s[i1, j0] + s[i0, j0]

        inter = box(s_pm)
        psum = box(s_p)
        msum = (
            sat_m[b][r1, c1] - sat_m[b][r0, c1]
            - sat_m[b][r1, c0] + sat_m[b][r0, c0]
        ).astype(np.float64)
        dice = (2.0 * inter + 1.0) / (psum + msum + 1.0)
        tot_sum += np.sum(1.0 - dice)
        tot_cnt += len(dice)

    l_comp = tot_sum / max(tot_cnt, 1) if tot_cnt > 0 else 0.0
    return np.array(bce + l_comp, dtype=np.float32)


# revision 15
# speedup vs baseline: 21.8935x; 1.1457x over previous
"""Trainium2 Bass kernel for nn_ComponentIoULoss.

Loss = global BCE (label smoothing, pos_weight=1) + mean per-connected-component
crop-dice over dilated component bboxes.

Device strategy (8 NeuronCores, data-parallel over batch, 2 images/core):
  - BCE partial sums on-device: sum(w*x) with w = 1 - smoothed(t), and
    sum(ln sigmoid(x))  (softplus(-x) = -ln sigmoid(x);
    BCE elem = (1-t')x + softplus(-x)).
  - Per-component crop dice needs box sums of p=sigmoid(x) and p*m over
    dilated component bboxes.  Targets are 8x8-blocky, so dilated bbox
    corners can only take 128 distinct values per axis:
    {0} + {8k-3 : k=1..63} + {8m+3 : m=1..63} + {512}.
    The device computes a "selected summed-area table": T1 = Lsel^T @ A
    (Lsel [512 x 128] prefix-indicator matrix, A in {p*m, p}), a cumulative
    sum scan along the free axis, packs the 128 selected columns on-chip and
    DMAs one contiguous SAT[128 x 128] per image per matrix.  The host reads
    4 SAT corners per component (exact prefix-sum differences).
  - Connected-component labeling (tiny integer problem, 64x64 coarse grid
    per image) and the final scalar combine run on the host.
"""

import sys

if "/opt/trn_rl_repo" not in sys.path:
    sys.path.insert(0, "/opt/trn_rl_repo")

import numpy as np
import ml_dtypes

N_CORES = 8
B, H, W = 16, 512, 512
BPC = B // N_CORES  # batches per core
BLK = 8
NC_COARSE = H // BLK  # 64
DIL = 3
EPS = 0.05
NSEL = 128

_COMPILED = {}


# ---------------------------------------------------------------------------
# selector geometry
# ---------------------------------------------------------------------------

def _sel_values():
    """The 128 prefix lengths v such that SAT[q] = sum over rows i < v[q]."""
    vs = [0]
    vs += [8 * k - 3 for k in range(1, 64)]
    vs += [8 * m + 3 for m in range(1, 64)]
    vs += [512]
    return np.array(vs, dtype=np.int64)


def _lo_index(r0):
    """Selector index for a box low edge r0 in {0} | {8k-3}."""
    return np.where(r0 == 0, 0, (r0 + 3) // 8)


def _hi_index(r1):
    """Selector index for a box high edge r1 in {8m+3} | {512}."""
    return np.where(r1 == 512, 127, 63 + (r1 - 3) // 8)


def _lsel_host():
    """Lsel^T [512 x 128] bf16 as [4, 128, 128] contract-chunk-major."""
    vs = _sel_values()
    i = np.arange(512)
    m = (i[:, None] < vs[None, :]).astype(ml_dtypes.bfloat16)
    return np.ascontiguousarray(m.reshape(4, 128, 128))


# ---------------------------------------------------------------------------
# host connected components (min-index propagation, mirrors reference)
# ---------------------------------------------------------------------------

def _cc_boxes(mask):
    """4-connected components of bool [h, w]. Returns (rmin, rmax, cmin, cmax)
    int arrays, one entry per component."""
    h, w = mask.shape
    n = h * w
    idx = np.arange(n, dtype=np.int64).reshape(h, w)
    lab = np.where(mask, idx, n)
    big = n
    while True:
        p = np.pad(lab, 1, constant_values=big)
        nmin = np.minimum(
            np.minimum(p[:-2, 1:-1], p[2:, 1:-1]),
            np.minimum(p[1:-1, :-2], p[1:-1, 2:]),
        )
        l2 = np.where(mask, np.minimum(lab, nmin), big)
        flat = np.append(l2.ravel(), big)
        flat = flat[flat]
        flat = flat[flat]
        l2 = flat[:-1].reshape(h, w)
        if np.array_equal(l2, lab):
            break
        lab = l2
    rows, cols = np.nonzero(mask)
    labs = lab[mask]
    uniq, inv = np.unique(labs, return_inverse=True)
    k = len(uniq)
    rmin = np.full(k, h, dtype=np.int64)
    rmax = np.full(k, -1, dtype=np.int64)
    cmin = np.full(k, w, dtype=np.int64)
    cmax = np.full(k, -1, dtype=np.int64)
    np.minimum.at(rmin, inv, rows)
    np.maximum.at(rmax, inv, rows)
    np.minimum.at(cmin, inv, cols)
    np.maximum.at(cmax, inv, cols)
    return rmin, rmax, cmin, cmax


# ---------------------------------------------------------------------------
# device kernel
# ---------------------------------------------------------------------------

def _build():
    import concourse.bacc as bacc
    import concourse.tile as tile
    from concourse import mybir
    from concourse.bass import _add_dep_helper

    F32 = mybir.dt.float32
    BF16 = mybir.dt.bfloat16
    AF = mybir.ActivationFunctionType
    OP = mybir.AluOpType

    nc = bacc.Bacc("TRN2", target_bir_lowering=False, debug=False,
                   num_devices=N_CORES)

    xbf_d = nc.dram_tensor("xbf", [BPC, H, W], BF16, kind="ExternalInput")
    tbf_d = nc.dram_tensor("tbf", [BPC, H, W], BF16, kind="ExternalInput")
    wbf_d = nc.dram_tensor("wbf", [BPC, H, W], BF16, kind="ExternalInput")
    lsel_d = nc.dram_tensor("lsel", [4, 128, 128], BF16, kind="ExternalInput")
    # single packed output: [sat_pm b0 | sat_p b0 | sat_pm b1 | sat_p b1 |
    #                        acc_ln b0 | acc_wx b0 | acc_ln b1 | acc_wx b1]
    out_d = nc.dram_tensor("out_all", [128, 2 * 128 * BPC + 2 * BPC], F32,
                           kind="ExternalOutput")

    with tile.TileContext(nc) as tc:
        with (
            tc.tile_pool(name="io", bufs=1) as io_pool,
            tc.tile_pool(name="mid", bufs=1) as mid_pool,
            tc.tile_pool(name="scr", bufs=1) as scr_pool,
            tc.tile_pool(name="scan", bufs=1) as scan_pool,
            tc.tile_pool(name="small", bufs=1) as small_pool,
            tc.tile_pool(name="ps", bufs=1, space="PSUM") as ps_pool,
        ):
            dummy = small_pool.tile([128, 512], F32, tag="dummy")
            nc.gpsimd.memset(dummy[:], 0.0)
            outt = small_pool.tile([128, 2 * 128 * BPC + 2 * BPC], F32,
                                   tag="outt")

            # lsel first: tiny, and it gates every matmul
            lsel = small_pool.tile([128, 4, 128], BF16, tag="lsel")
            nc.sync.dma_start(
                lsel[:], lsel_d.ap().rearrange("c p q -> p c q")
            )
            # x arrives in 4 row-chunks per image so sigmoid/a1/matmul start
            # as soon as the first 128 rows land
            xs, tbs, wbs = [], [], []
            for b in range(BPC):
                x = io_pool.tile([128, 2048], BF16, tag=f"x{b}")
                x3 = x[:].rearrange("p (c w) -> p c w", w=W)
                for c in range(4):
                    nc.sync.dma_start(
                        x3[:, c, :],
                        xbf_d.ap()[b, c * 128:(c + 1) * 128, :],
                    )
                tb = io_pool.tile([128, 2048], BF16, tag=f"t{b}")
                nc.sync.dma_start(
                    tb[:].rearrange("p (c w) -> p c w", w=W),
                    tbf_d.ap()[b].rearrange("(c p) w -> p c w", p=128),
                )
                wb = io_pool.tile([128, 2048], BF16, tag=f"w{b}")
                nc.sync.dma_start(
                    wb[:].rearrange("p (c w) -> p c w", w=W),
                    wbf_d.ap()[b].rearrange("(c p) w -> p c w", p=128),
                )
                xs.append(x)
                tbs.append(tb)
                wbs.append(wb)

            # --- critical path: sigmoid -> a1 -> matmul -> scan -> pack ---
            p_bfs, a1s, sig_insts = [], [], []
            for b in range(BPC):
                x3 = xs[b][:].rearrange("p (c w) -> p c w", w=W)
                tb3 = tbs[b][:].rearrange("p (c w) -> p c w", w=W)
                p_bf = mid_pool.tile([128, 2048], BF16, tag=f"p{b}")
                p3 = p_bf[:].rearrange("p (c w) -> p c w", w=W)
                a1 = mid_pool.tile([128, 2048], BF16, tag=f"a1{b}")
                a13 = a1[:].rearrange("p (c w) -> p c w", w=W)
                t1_pm = ps_pool.tile([128, 512], F32, tag=f"t1pm{b}")
                t1_p = ps_pool.tile([128, 512], F32, tag=f"t1p{b}")
                for c in range(4):
                    sig_insts.append(
                        nc.scalar.activation(p3[:, c, :], x3[:, c, :],
                                             AF.Sigmoid)
                    )
                    nc.tensor.matmul(
                        t1_p[:], lsel[:, c, :], p3[:, c, :],
                        start=(c == 0), stop=(c == 3),
                    )
                    nc.vector.tensor_tensor(a13[:, c, :], p3[:, c, :],
                                            tb3[:, c, :], OP.mult)
                    nc.tensor.matmul(
                        t1_pm[:], lsel[:, c, :], a13[:, c, :],
                        start=(c == 0), stop=(c == 3),
                    )
                p_bfs.append(p_bf)
                a1s.append(a1)
                for mi, t1 in enumerate((t1_pm, t1_p)):
                    name = ("pm", "p")[mi]
                    scan = scan_pool.tile([128, 512], F32, tag=f"scan{name}{b}")
                    nc.vector.tensor_tensor_scan(
                        scan[:], t1[:], dummy[:], 0.0, OP.add, OP.add
                    )
                    # pack the 128 selected columns (col 0 = empty prefix is
                    # zeroed host-side) into the combined output tile;
                    # GpSimd so the DVE tail stays free for the scans
                    base = (2 * b + mi) * 128
                    nc.gpsimd.tensor_copy(outt[:, base + 1:base + 64],
                                          scan[:, 4:508:8])
                    nc.gpsimd.tensor_copy(outt[:, base + 64:base + 127],
                                          scan[:, 10:512:8])
                    nc.gpsimd.tensor_copy(outt[:, base + 127:base + 128],
                                          scan[:, 511:512])

            # --- BCE partial sums (fill engine gaps; issued last) ---
            for b in range(BPC):
                acc_ln = small_pool.tile([128, 1], F32, tag=f"acc_ln{b}")
                lnscr = scr_pool.tile([128, 2048], BF16, tag=f"lnscr{b}")
                ln_inst = nc.scalar.activation(
                    lnscr[:], p_bfs[b][:], AF.Ln, accum_out=acc_ln[:]
                )
                # keep ACT ordered sigmoid*8, ln, ln -> 2 act-table loads
                _add_dep_helper(ln_inst.ins, sig_insts[-1].ins, sync=False,
                                reason="group act tables")
                nc.vector.tensor_copy(outt[:, 512 + 2 * b:513 + 2 * b],
                                      acc_ln[:])

                # wx = w * x on GpSimd (otherwise idle); sum on ACT
                # (Identity is in every act-table set: no table reload)
                wx = scr_pool.tile([128, 2048], BF16, tag=f"wx{b}")
                nc.gpsimd.tensor_tensor(wx[:], wbs[b][:], xs[b][:], OP.mult)
                acc_wx = small_pool.tile([128, 1], F32, tag=f"acc_wx{b}")
                wxscr = scr_pool.tile([128, 2048], BF16, tag=f"wxscr{b}")
                nc.scalar.activation(
                    wxscr[:], wx[:], AF.Identity, accum_out=acc_wx[:]
                )
                nc.vector.tensor_copy(outt[:, 513 + 2 * b:514 + 2 * b],
                                      acc_wx[:])

            nc.sync.dma_start(out_d.ap(), outt[:])

    nc.compile()
    return nc


def _get_compiled():
    if "nc" not in _COMPILED:
        _COMPILED["nc"] = _build()
    return _COMPILED["nc"]


def _in_maps(logit, target):
    """Shard full [B,1,H,W] f32 inputs into per-core input maps."""
    lsel = _lsel_host()
    xbf = logit[:, 0].astype(ml_dtypes.bfloat16)
    tbf = target[:, 0].astype(ml_dtypes.bfloat16)
    wbf = ((1.0 - EPS) - (1.0 - 2.0 * EPS) * target[:, 0]).astype(
        ml_dtypes.bfloat16
    )
    return [
        {
            "xbf": np.ascontiguousarray(xbf[c * BPC:(c + 1) * BPC]),
            "tbf": np.ascontiguousarray(tbf[c * BPC:(c + 1) * BPC]),
            "wbf": np.ascontiguousarray(wbf[c * BPC:(c + 1) * BPC]),
            "lsel": lsel,
        }
        for c in range(N_CORES)
    ]


# ---------------------------------------------------------------------------
# host fallback (general inputs; not hit for blocky targets)
# ---------------------------------------------------------------------------

def _host_reference(logit, target):
    x = logit[:, 0].astype(np.float64)
    t = target[:, 0].astype(np.float64)
    ts = t * (1.0 - EPS) + (1.0 - t) * EPS
    bce = np.mean((1.0 - ts) * x + np.logaddexp(0.0, -x))
    p = 1.0 / (1.0 + np.exp(-x))
    tot_sum = 0.0
    tot_cnt = 0
    for b in range(B):
        mask = t[b] > 0.5
        if not mask.any():
            continue
        rmin, rmax, cmin, cmax = _cc_boxes(mask)
        r0 = np.clip(rmin - DIL, 0, H)
        r1 = np.clip(rmax + 1 + DIL, 0, H)
        c0 = np.clip(cmin - DIL, 0, W)
        c1 = np.clip(cmax + 1 + DIL, 0, W)
        sat_pm = np.zeros((H + 1, W + 1))
        sat_pm[1:, 1:] = (p[b] * t[b]).cumsum(0).cumsum(1)
        sat_p = np.zeros((H + 1, W + 1))
        sat_p[1:, 1:] = p[b].cumsum(0).cumsum(1)
        sat_m = np.zeros((H + 1, W + 1))
        sat_m[1:, 1:] = t[b].cumsum(0).cumsum(1)

        def box(s):
            return s[r1, c1] - s[r0, c1] - s[r1, c0] + s[r0, c0]

        dice = (2.0 * box(sat_pm) + 1.0) / (box(sat_p) + box(sat_m) + 1.0)
        tot_sum += np.sum(1.0 - dice)
        tot_cnt += len(dice)
    l_comp = tot_sum / max(tot_cnt, 1) if tot_cnt > 0 else 0.0
    return np.array(bce + l_comp, dtype=np.float32)


# ---------------------------------------------------------------------------
# entry point
# ---------------------------------------------------------------------------

def kernel(logit, target):
    from concourse.bass_utils import run_bass_kernel_spmd

    logit = np.ascontiguousarray(np.asarray(logit, dtype=np.float32))
    target = np.ascontiguousarray(np.asarray(target, dtype=np.float32))
    assert logit.shape == (B, 1, H, W) and target.shape == (B, 1, H, W)

    tgt = target[:, 0]
    coarse = tgt[:, ::BLK, ::BLK]
    blocky = np.array_equal(
        tgt.reshape(B, NC_COARSE, BLK, NC_COARSE, BLK),
        np.broadcast_to(
            coarse[:, :, None, :, None],
            (B, NC_COARSE, BLK, NC_COARSE, BLK),
        ),
    )
    if not blocky:
        return _host_reference(logit, target)

    nc = _get_compiled()
    res = run_bass_kernel_spmd(
        nc, _in_maps(logit, target), core_ids=list(range(N_CORES))
    )

    # ---- BCE ----
    s_ln = 0.0
    s_wx = 0.0
    for c in range(N_CORES):
        out = res.results[c]["out_all"].astype(np.float64)
        for b in range(BPC):
            s_ln += out[:, 512 + 2 * b].sum()
            s_wx += out[:, 513 + 2 * b].sum()
    n_tot = float(B * H * W)
    # sum softplus(-x) = -sum ln p ; bce elem = w*x + softplus(-x)
    bce = (s_wx - s_ln) / n_tot

    # ---- per-component crop dice ----
    tm = tgt > 0.5
    sat_m = np.zeros((B, H + 1, W + 1), dtype=np.int64)
    sat_m[:, 1:, 1:] = tm.astype(np.int64).cumsum(1).cumsum(2)

    tot_sum = 0.0
    tot_cnt = 0
    for b in range(B):
        cm = coarse[b] > 0.5
        if not cm.any():
            continue
        crmin, crmax, ccmin, ccmax = _cc_boxes(cm)
        r0 = np.maximum(BLK * crmin - DIL, 0)
        r1 = np.minimum(BLK * (crmax + 1) + DIL, H)
        c0 = np.maximum(BLK * ccmin - DIL, 0)
        c1 = np.minimum(BLK * (ccmax + 1) + DIL, W)

        core, slot = b // BPC, b % BPC
        out = res.results[core]["out_all"]
        s_pm = out[:, (2 * slot) * 128:(2 * slot) * 128 + 128].astype(np.float64)
        s_p = out[:, (2 * slot + 1) * 128:(2 * slot + 1) * 128 + 128].astype(np.float64)
        s_pm[:, 0] = 0.0
        s_pm[0, :] = 0.0
        s_p[:, 0] = 0.0
        s_p[0, :] = 0.0

        i0 = _lo_index(r0)
        i1 = _hi_index(r1)
        j0 = _lo_index(c0)
        j1 = _hi_index(c1)

        def box(s):
            return s[i1, j1] - s[i0, j1] - s[i1, j0] + s[i0, j0]

        inter = box(s_pm)
        psum = box(s_p)
        msum = (
            sat_m[b][r1, c1] - sat_m[b][r0, c1]
            - sat_m[b][r1, c0] + sat_m[b][r0, c0]
        ).astype(np.float64)
        dice = (2.0 * inter + 1.0) / (psum + msum + 1.0)
        tot_sum += np.sum(1.0 - dice)
        tot_cnt += len(dice)

    l_comp = tot_sum / max(tot_cnt, 1) if tot_cnt > 0 else 0.0
    return np.array(bce + l_comp, dtype=np.float32)
